# revision 9
# baseline (speedup 1.0000x reference)
"""Trainium2 kernel for: out = tanh(x @ scatter_nd(nonzero_ind, kernel_vector, (20000, 4096)) + bias).

Strategy (8 NeuronCores), W-resident / x-streaming, units sharded x8:
  core c owns W[:, c*512:(c+1)*512] (20096 x 512 fp16, SBUF-resident) and
  computes out[:, c*512:(c+1)*512] = x @ W_c for the full batch.

v3 (trace-driven, vs 569us baseline):
  - ALL DMA traffic rides the two HWDGE rings (sync/scalar), ~115 GB/s
    each during chunk 0 -- the SWDGE (gpsimd) queue degrades to ~70 GB/s
    under fabric load (descriptor rings live in SBUF and contend).
    512 KB transfers: x as k-tile pairs, W as 4-k-tile groups.
  - Ring-FIFO aware issue order: x pair 0/1 issued before the W prologue
    (head-of-line), W paced 4 groups ahead of use in the k-loop.
  - Next chunk's first x pairs are issued BEFORE the drain casts so they
    don't queue behind cast sem-waits on the same engine streams.
  - PE warmup: memset + 8 garbage matmuls cover the first-DMA wait and
    the HAM cold-clock window.
  - Chunks [1024, 512, 512]: PSUM pool A holds s=0 banks, pool B s=1;
    512-chunks alternate A/B so each boundary only waits the first bank
    set, casts split across Vector AND Scalar engines, one consolidated
    stage tile + out-DMA halves per chunk -> ~3.5us tail.
"""

import numpy as np

P = 128
B, K, U = 2048, 20000, 4096
USPLIT = 8
KT = 157                 # k-tiles (full contraction per core)
KTP = 158                # padded to even for x k-tile pairs
KPAD_X = KTP * P         # 20224 rows (224 zero pad) for x packing
U_SH = U // USPLIT       # 512 unit cols per core
NUS = U_SH // P          # 4 W subtiles (stationary blocks) per k-tile
NXP = KTP // 2           # 79 x k-tile pairs
NWG = 40                 # W 4-k-tile groups (last group: 1 real k-tile)
KPAD_W = NWG * 4 * P     # 20480 rows for W group packing
XCOLS = 2048             # uniform x pool tile cols (max chunk: 2*1024)

# chunk config: list of (batch_size, n_batch_blocks); BBLK = size // nbb = 512
CHUNKS = [(1024, 2), (512, 1), (512, 1)]
BBLK = 512

TRACE = False            # set by test harness for profiled runs
LAST_RESULT = None       # BassKernelResults of the last run (for the harness)

_NC_CACHE = {}


def _build_nc():
    from concourse import bacc
    import concourse.mybir as mybir
    import concourse.tile as tile

    f32 = mybir.dt.float32
    f16 = mybir.dt.float16
    bf16 = mybir.dt.bfloat16

    nc = bacc.Bacc("TRN2", target_bir_lowering=False, debug=False)

    # x^T k-tile pairs per chunk: xt{ch}[pair, p, j*bch + b]
    #   = x[b0(ch) + b, (2*pair+j)*128 + p]  (fp16, zero-padded rows)
    xt_d = [
        nc.dram_tensor(f"xt{ch}", [NXP, P, 2 * bch], f16, kind="ExternalInput").ap()
        for ch, (bch, nbb) in enumerate(CHUNKS)
    ]
    # W 4-k-tile groups: w[g, p, j*512 + u] = W[(4g+j)*128 + p, u]
    w_d = nc.dram_tensor("w_sh", [NWG, P, 4 * U_SH], f16, kind="ExternalInput").ap()
    # out per chunk: o{ch}[p, us*nbb*BBLK + s*BBLK + b] = z^T[us*128+p, ...]
    o_d = [
        nc.dram_tensor(f"o{ch}", [P, NUS, nbb * BBLK], bf16,
                       kind="ExternalOutput").ap()
        for ch, (bch, nbb) in enumerate(CHUNKS)
    ]

    with tile.TileContext(nc) as tc:
        with (
            tc.tile_pool(name="resid", bufs=1) as respool,
            tc.tile_pool(name="xpool", bufs=6) as xpool,
            tc.tile_pool(name="stage", bufs=1) as spool,
            tc.tile_pool(name="warm", bufs=1) as wmpool,
            tc.tile_pool(name="psumA", bufs=1, space="PSUM") as psumA,
            tc.tile_pool(name="psumB", bufs=1, space="PSUM") as psumB,
        ):
            xq = [nc.sync, nc.scalar]
            prefetched = {}

            def x_issue(ch, pair):
                t = xpool.tile([P, XCOLS], f16, tag="xs", name="xs")
                bch = CHUNKS[ch][0]
                xq[pair % 2].dma_start(t[:, :2 * bch], xt_d[ch][pair])
                prefetched[(ch, pair)] = t

            # --- PE warmup: keep the PE busy through the HAM cold window
            # while the first x/W DMAs are in flight.  Garbage matmuls into
            # the slot chunk 0 will overwrite (start=True clears the bank).
            wtile = wmpool.tile([P, BBLK], f16, tag="wm", name="wm")
            nc.vector.memset(wtile[:], 0.0)
            wps = psumA.tile([P, BBLK], f32, tag="pa0", name="wps")

            # x pairs 0/1 first on each ring (ring FIFO: first issued =
            # first completed), then the W prologue.
            x_issue(0, 0)
            x_issue(0, 1)

            wres = [
                respool.tile([P, 4 * U_SH], f16, tag=f"w{g}", name=f"w{g}")
                for g in range(NWG)
            ]

            def w_dma(g):
                weng = nc.sync if g % 2 == 0 else nc.scalar
                weng.dma_start(wres[g][:], w_d[g])

            for g in range(4):       # prologue: first 4 groups
                w_dma(g)

            for _ in range(8):
                nc.tensor.matmul(wps[:], wtile[:, 0:P], wtile[:],
                                 start=True, stop=True)

            n512 = 0
            for ch, (bch, nbb) in enumerate(CHUNKS):
                # PSUM: s=0 blocks in pool A, s=1 in pool B (nbb=2);
                # nbb=1 chunks alternate pools A, B, A, ...
                if nbb == 2:
                    pools = [(psumA, "a"), (psumB, "b")]
                else:
                    pools = [[(psumA, "a"), (psumB, "b")][n512 % 2]]
                    n512 += 1
                psums = [
                    [pool.tile([P, BBLK], f32, tag=f"p{pc}{us}",
                               name=f"ps{ch}_{us}_{s}")
                     for s, (pool, pc) in enumerate(pools)]
                    for us in range(NUS)
                ]

                for pair in range(NXP):
                    if (ch, pair) not in prefetched:
                        x_issue(ch, pair)
                    xs = prefetched.pop((ch, pair))
                    for j in range(2):
                        kt = 2 * pair + j
                        if kt >= KT:
                            break
                        if ch == 0 and kt % 4 == 0 and kt // 4 + 4 < NWG:
                            w_dma(kt // 4 + 4)   # paced W prefetch
                        g, jj = kt // 4, kt % 4
                        # first k-tile after a boundary: touch the banks in
                        # the order the previous chunk's casts free them
                        us_order = [0, 2, 1, 3] if (kt == 0 and ch > 0) \
                            else range(NUS)
                        for us in us_order:
                            for s in range(nbb):
                                nc.tensor.matmul(
                                    psums[us][s][:],
                                    wres[g][:, jj * U_SH + us * P:
                                            jj * U_SH + (us + 1) * P],
                                    xs[:, j * bch + s * BBLK:
                                           j * bch + (s + 1) * BBLK],
                                    start=(kt == 0),
                                    stop=(kt == KT - 1),
                                )

                # Prefetch next chunk's first x pairs BEFORE the drain --
                # the casts below block the engine streams on matmul sems.
                if ch + 1 < len(CHUNKS):
                    for pp in range(3):
                        x_issue(ch + 1, pp)

                # Drain: cast PSUM -> one stage tile; vector does us 0-1,
                # scalar does us 2-3 in parallel; pool-A banks (which the
                # next chunk needs first) are cast before pool-B banks.
                st = spool.tile([P, NUS * nbb * BBLK], bf16,
                                tag=f"st{ch}", name=f"st{ch}")
                for s in range(nbb):          # s=0 (pool A) first
                    for us in range(NUS):
                        dst = st[:, (us * nbb + s) * BBLK:
                                 (us * nbb + s + 1) * BBLK]
                        if us < 2:
                            nc.vector.tensor_copy(dst, psums[us][s][:])
                        else:
                            nc.scalar.copy(dst, psums[us][s][:])
                # out-DMA halves: us 0-1 on sync after vector's casts,
                # us 2-3 on scalar after its own casts.
                half = 2 * nbb * BBLK
                nc.sync.dma_start(o_d[ch][:, 0:2], st[:, :half])
                nc.scalar.dma_start(o_d[ch][:, 2:4], st[:, half:])

    nc.compile()
    return nc


def _get_nc():
    if "nc" not in _NC_CACHE:
        _NC_CACHE["nc"] = _build_nc()
    return _NC_CACHE["nc"]


def kernel(x, kernel_vector, bias, nonzero_ind):
    global LAST_RESULT
    from concourse.bass_utils import run_bass_kernel_spmd

    x = np.asarray(x, dtype=np.float32)
    kernel_vector = np.asarray(kernel_vector, dtype=np.float32)
    bias = np.asarray(bias, dtype=np.float32)
    nonzero_ind = np.asarray(nonzero_ind)

    nc = _get_nc()

    # Host scatter: dense weights [KPAD_W, U] fp16 (rows >= 20000 stay zero).
    rows = nonzero_ind[:, 0].astype(np.int64)
    cols = nonzero_ind[:, 1].astype(np.int64)
    w_full = np.zeros(KPAD_W * U, np.float32)
    np.add.at(w_full, rows * U + cols, kernel_vector)
    w_full = w_full.reshape(KPAD_W, U).astype(np.float16)

    # x^T padded to KPAD_X rows, fp16.
    x16 = x.astype(np.float16)
    xt = np.zeros((KPAD_X, B), np.float16)
    xt[:K] = x16.T

    # Per-chunk pair layout: xt{ch}[pair, p, j*bch + b] = xt[(2p+j)*128+p, b0+b]
    xt_chunks = []
    b0 = 0
    for bch, nbb in CHUNKS:
        xc = xt[:, b0:b0 + bch].reshape(NXP, 2, P, bch).transpose(0, 2, 1, 3)
        xt_chunks.append(np.ascontiguousarray(xc.reshape(NXP, P, 2 * bch)))
        b0 += bch

    in_maps = []
    for c in range(USPLIT):
        # W groups: w[g, p, j*512 + u] = W[(4g+j)*128 + p, c*512 + u]
        w_sh = w_full[:, c * U_SH:(c + 1) * U_SH]
        w_sh = w_sh.reshape(NWG, 4, P, U_SH).transpose(0, 2, 1, 3)
        w_sh = np.ascontiguousarray(w_sh.reshape(NWG, P, 4 * U_SH))
        m = {"w_sh": w_sh}
        for ch in range(len(CHUNKS)):
            m[f"xt{ch}"] = xt_chunks[ch]
        in_maps.append(m)

    kwargs = {}
    if TRACE:
        kwargs = dict(trace=True, trace_cores=list(range(8)))
    res = run_bass_kernel_spmd(nc, in_maps, core_ids=list(range(8)), **kwargs)
    LAST_RESULT = res

    out = np.empty((B, U), np.float32)
    for c in range(USPLIT):
        b0 = 0
        for ch, (bch, nbb) in enumerate(CHUNKS):
            # [P, NUS, nbb*BBLK] -> [nbb, BBLK, NUS, P] -> [bch, U_SH]
            blk = (
                res.results[c][f"o{ch}"]
                .astype(np.float32)
                .reshape(P, NUS, nbb, BBLK)
                .transpose(2, 3, 1, 0)
                .reshape(bch, U_SH)
            )
            out[b0:b0 + bch, c * U_SH:(c + 1) * U_SH] = blk
            b0 += bch
    out += bias[None, :]
    np.tanh(out, out=out)
    return out
